# revision 1
# baseline (speedup 1.0000x reference)
"""Trainium2 Bass kernel for nn_FuncSelfAttention (spectral self-attention).

Math: the spectral convs keep only 2x2 Fourier modes, so rfft2/irfft2 collapse
to a [1024 -> 8] projection (E8) and an [8 -> 1024] reconstruction (Bas).  The
whole network runs in the 8-dim mode/coefficient space; attention inner
products over (hd, H, W) reduce to a diagonal 8x8 Gram matrix.  The only large
data movement is reading seq (128 MiB) and writing the output (128 MiB) =>
memory-bound.  Sharding: data-parallel over batch (B=8) across 8 cores.

Per core: x [4096=(s,c), 1024=(h,w)] ->
  stage 1: PE-transpose x chunks, project with E8 -> mode coords XR/XI [c,(m,s)]
  phase 2: complex channel mixing (w_qkv) -> Q/K/V coeffs [s, (jm,h,d)]
  attn:    per head: scores = (g-scaled Uq)^T Uk + cpb bias, softmax, attn @ V
  phase 6: T8-diag scale + w_out mixing -> final coeffs U_fT [8, rows]
  stage 7: y = U_fT^T @ Bas -> [4096, 1024] streamed out.
"""
import numpy as np

B, S, C, H, W = 8, 64, 64, 32, 32
NH, HD = 8, 8
HW = H * W
NCORES = 8
MODES4 = [(0, 0), (0, 1), (1, 0), (1, 1)]

# DT_BIG: dtype for the two big matmul paths (stage 1 projection, stage 7
# reconstruction) and their transposed operands.  "f32r" = fp32-replicated
# (full-rate on PE), "bf16", or "f32" (exact, 4x slower PE).
DT_BIG = "f16"


def _constants():
    hh, ww = np.meshgrid(np.arange(H), np.arange(W), indexing="ij")
    phi, psi = 2 * np.pi / H, 2 * np.pi / W
    E8 = np.zeros((HW, 8))
    Bas = np.zeros((8, HW))
    for mi, (kx, ky) in enumerate(MODES4):
        th = phi * kx * hh + psi * ky * ww
        E8[:, 2 * mi] = np.cos(th).ravel()
        E8[:, 2 * mi + 1] = -np.sin(th).ravel()
        mult = 1.0 if ky == 0 else 2.0
        Bas[2 * mi] = mult / HW * np.cos(th).ravel()
        Bas[2 * mi + 1] = -mult / HW * np.sin(th).ravel()
    g = (Bas @ Bas.T).diagonal().copy()      # attention Gram diag
    t8d = (Bas @ E8).diagonal().copy()       # coeff->mode map (diagonal)

    e8c = np.zeros((128, 64), np.float32)    # chunk k at cols [8k, 8k+8)
    for k in range(8):
        e8c[:, 8 * k:8 * k + 8] = E8[128 * k:128 * (k + 1)]

    gx, gy = np.meshgrid(np.arange(8), np.arange(8), indexing="ij")
    coords = np.stack([gx.ravel(), gy.ravel()], -1).astype(np.float32)
    rel = coords[:, None, :] - coords[None, :, :]
    rel = np.sign(rel) * np.log2(1.0 + np.abs(rel))          # [64, 64, 2]
    relT = np.ascontiguousarray(rel.reshape(4096, 2).T).astype(np.float32)

    scale = np.float32(1.0 / HW) / np.float32(np.sqrt(HD))
    gcol = np.zeros((64, 1), np.float32)     # dj order = (jm, d): p = jm*8+d
    for p in range(64):
        gcol[p, 0] = g[p // 8] * scale
    t8pat = np.zeros((1, 512), np.float32)   # over (jm, h, d): col = jm*64+..
    for jm in range(8):
        t8pat[0, jm * 64:(jm + 1) * 64] = t8d[jm]
    return e8c, Bas.astype(np.float32), relT, gcol, t8pat


def _build(dt_big_name=DT_BIG):
    import concourse.bass as bass
    import concourse.mybir as mybir
    import concourse.tile as tile
    from concourse import bacc
    from concourse.masks import make_identity

    f32 = mybir.dt.float32
    dt_big = {"f32r": mybir.dt.float32r, "bf16": mybir.dt.bfloat16,
              "f16": mybir.dt.float16, "f32": f32}[dt_big_name]
    cast_on_load = dt_big_name in ("bf16", "f16")
    Exp = mybir.ActivationFunctionType.Exp
    Relu = mybir.ActivationFunctionType.Relu

    nc = bacc.Bacc("TRN2", target_bir_lowering=False, debug=False)
    x_in = nc.dram_tensor("x", [4096, 1024], f32, kind="ExternalInput")
    wqr_in = nc.dram_tensor("wqr", [64, 768], f32, kind="ExternalInput")
    wqi_in = nc.dram_tensor("wqi", [64, 768], f32, kind="ExternalInput")
    wor_in = nc.dram_tensor("wor", [64, 256], f32, kind="ExternalInput")
    woi_in = nc.dram_tensor("woi", [64, 256], f32, kind="ExternalInput")
    cw1_in = nc.dram_tensor("cw1", [2, 64], f32, kind="ExternalInput")
    cb1_in = nc.dram_tensor("cb1", [64, 1], f32, kind="ExternalInput")
    cw2_in = nc.dram_tensor("cw2", [64, 8], f32, kind="ExternalInput")
    e8_in = nc.dram_tensor("e8c", [128, 64], f32, kind="ExternalInput")
    bas_in = nc.dram_tensor("bas", [8, 1024], f32, kind="ExternalInput")
    rel_in = nc.dram_tensor("relT", [2, 4096], f32, kind="ExternalInput")
    gcol_in = nc.dram_tensor("gcol", [64, 1], f32, kind="ExternalInput")
    t8_in = nc.dram_tensor("t8pat", [1, 512], f32, kind="ExternalInput")
    y_out = nc.dram_tensor("y", [4096, 1024], f32, kind="ExternalOutput")

    with tile.TileContext(nc) as tc:
        import contextlib
        ctx = contextlib.ExitStack()
        with ctx:
            singles = ctx.enter_context(tc.tile_pool(name="singles", bufs=1))
            ps = ctx.enter_context(tc.tile_pool(name="ps", bufs=5, space="PSUM"))
            psl = ctx.enter_context(tc.tile_pool(name="psl", bufs=1, space="PSUM"))
            x_pool = ctx.enter_context(tc.tile_pool(name="xp", bufs=4))
            xt_pool = ctx.enter_context(tc.tile_pool(name="xt", bufs=3))
            m_pool = ctx.enter_context(tc.tile_pool(name="mp", bufs=3))
            sm_pool = ctx.enter_context(tc.tile_pool(name="sm", bufs=4))
            y_pool = ctx.enter_context(tc.tile_pool(name="yp", bufs=4))

            # ---- constants / weights into SBUF ----
            def load1(name, dram, shape):
                t = singles.tile(shape, f32, tag=name)
                nc.sync.dma_start(out=t[:], in_=dram[:])
                return t

            e8_f = load1("e8", e8_in, [128, 64])
            bas_f = load1("bas", bas_in, [8, 1024])
            relT = load1("relT", rel_in, [2, 4096])
            gcol = load1("gcol", gcol_in, [64, 1])
            wqr = load1("wqr", wqr_in, [64, 768])
            wqi = load1("wqi", wqi_in, [64, 768])
            wor = load1("wor", wor_in, [64, 256])
            woi = load1("woi", woi_in, [64, 256])
            cw1 = load1("cw1", cw1_in, [2, 64])
            cb1 = load1("cb1", cb1_in, [64, 1])
            cw2 = load1("cw2", cw2_in, [64, 8])
            t8rep = singles.tile([64, 512], f32, tag="t8rep")
            nc.sync.dma_start(out=t8rep[:], in_=t8_in[:].to_broadcast([64, 512]))

            ident = singles.tile([128, 128], f32, tag="ident")
            make_identity(nc, ident[:])

            dt_mid = dt_big if dt_big_name == "f16" else f32
            wqrm = singles.tile([64, 768], dt_mid, tag="wqrm")
            nc.vector.tensor_copy(wqrm[:], wqi[:]) if False else nc.vector.tensor_copy(wqrm[:], wqr[:])
            wqim = singles.tile([64, 768], dt_mid, tag="wqim")
            nc.vector.tensor_copy(wqim[:], wqi[:])
            wqin = singles.tile([64, 768], dt_mid, tag="wqin")
            nc.vector.tensor_scalar_mul(wqin[:], wqi[:], -1.0)
            worm = singles.tile([64, 256], dt_mid, tag="worm")
            nc.vector.tensor_copy(worm[:], wor[:])
            woim = singles.tile([64, 256], dt_mid, tag="woim")
            nc.vector.tensor_copy(woim[:], woi[:])
            woin = singles.tile([64, 256], dt_mid, tag="woin")
            nc.vector.tensor_scalar_mul(woin[:], woi[:], -1.0)

            # big-path operands in dt_big
            if dt_big != f32:
                e8b = singles.tile([128, 64], dt_big, tag="e8b")
                nc.vector.tensor_copy(e8b[:], e8_f[:])
                basb = singles.tile([8, 1024], dt_big, tag="basb")
                nc.vector.tensor_copy(basb[:], bas_f[:])
                identb = singles.tile([128, 128], dt_big, tag="identb")
                nc.vector.tensor_copy(identb[:], ident[:])
            else:
                e8b, basb, identb = e8_f, bas_f, ident

            # persistent intermediates
            XR = singles.tile([64, 256], dt_mid, tag="XR")   # [c, (m, s)]
            XI = singles.tile([64, 256], dt_mid, tag="XI")
            h_relu = singles.tile([64, 4096], dt_mid, tag="hrelu")
            bias_sb = singles.tile([64, 512], f32, tag="bias")   # [i, (h, j)]
            Q_sb = singles.tile([64, 512], f32, tag="Qsb")  # [s, (jm, h, d)]
            K_sb = singles.tile([64, 512], f32, tag="Ksb")
            V_sb = singles.tile([64, 512], f32, tag="Vsb")
            O_all = singles.tile([64, 512], f32, tag="Oall")  # [i, (jm, h, d)]
            O_sc = singles.tile([64, 512], f32, tag="Osc")
            XOR = singles.tile([64, 256], dt_mid, tag="XOR")  # [c, (m, s)]
            XOI = singles.tile([64, 256], dt_mid, tag="XOI")
            F_sb = singles.tile([64, 512], f32, tag="Fsb")   # [c_out, (jm, s)]
            U_fT = singles.tile([8, 4096], dt_big, tag="UfT")  # [jm, rows]

            # ---- stage 1: transpose + project; 8 groups of 512 rows ----
            xt_dt = dt_big if cast_on_load else f32
            tr_ident = identb if cast_on_load else ident
            for gi in range(8):
                xTg = xt_pool.tile([128, 4096], dt_big, tag="xTg")
                xTg4 = xTg.rearrange("p (k t r) -> p k t r", k=8, t=4)
                for t in range(4):
                    r0 = 512 * gi + 128 * t
                    x_t = x_pool.tile([128, 1024], xt_dt, tag="x_t")
                    if cast_on_load:
                        nc.gpsimd.dma_start(out=x_t[:], in_=x_in[r0:r0 + 128, :])
                    else:
                        nc.sync.dma_start(out=x_t[:], in_=x_in[r0:r0 + 128, :])
                    for a in range(2):
                        ptr = ps.tile([128, 512], xt_dt, tag="ps")
                        for j in range(4):
                            k = 4 * a + j
                            nc.tensor.transpose(ptr[:, 128 * j:128 * (j + 1)],
                                                x_t[:, 128 * k:128 * (k + 1)],
                                                tr_ident[:])
                        dst = xTg4[:, 4 * a:4 * a + 4, t, :]
                        if (t + a) % 2 == 0:
                            nc.vector.tensor_copy(dst, ptr[:])
                        else:
                            nc.scalar.copy(dst, ptr[:])
                pm = ps.tile([8, 512], f32, tag="ps")
                for k in range(8):
                    nc.tensor.matmul(pm[:], e8b[:, 8 * k:8 * k + 8],
                                     xTg[:, 512 * k:512 * (k + 1)],
                                     start=(k == 0), stop=(k == 7))
                m_sb = m_pool.tile([8, 512], dt_mid, tag="m_sb")
                nc.vector.tensor_copy(m_sb[:], pm[:])
                # per-s transposes [8, 64] -> [64, 8], all into one psum [64, 64]
                pxg = ps.tile([64, 64], dt_mid, tag="ps")
                tid = identb if dt_mid != f32 else ident
                for u in range(8):
                    nc.tensor.transpose(pxg[:, 8 * u:8 * u + 8],
                                        m_sb[:, 64 * u:64 * (u + 1)], tid[:8, :8])
                # scatter to XR/XI: src (c, u, m, t) -> dst (c, m, s=8g+u)
                pxv = pxg.rearrange("c (u m t) -> c m u t", m=4, t=2)
                xr3 = XR.rearrange("c (m s) -> c m s", s=64)
                xi3 = XI.rearrange("c (m s) -> c m s", s=64)
                nc.vector.tensor_copy(xr3[:, :, 8 * gi:8 * gi + 8], pxv[:, :, :, 0])
                nc.vector.tensor_copy(xi3[:, :, 8 * gi:8 * gi + 8], pxv[:, :, :, 1])

            # ---- CPB bias: relu(relT^T @ cw1 + b1) @ cw2 -> [i, (h, j)] ----
            if dt_big != f32:
                relTb = singles.tile([2, 4096], dt_big, tag="relTb")
                nc.vector.tensor_copy(relTb[:], relT[:])
                cw1b = singles.tile([2, 64], dt_big, tag="cw1b")
                nc.vector.tensor_copy(cw1b[:], cw1[:])
            else:
                relTb, cw1b = relT, cw1
            for n in range(8):
                pc = ps.tile([64, 512], f32, tag="ps")
                nc.tensor.matmul(pc[:], cw1b[:], relTb[:, 512 * n:512 * (n + 1)],
                                 start=True, stop=True)
                nc.scalar.activation(h_relu[:, 512 * n:512 * (n + 1)], pc[:],
                                     Relu, bias=cb1[:])
            cw2m = singles.tile([64, 8], dt_mid, tag="cw2m")
            nc.vector.tensor_copy(cw2m[:], cw2[:])
            h3 = h_relu.rearrange("e (i j) -> e i j", j=64)
            b3 = bias_sb.rearrange("i (h j) -> i h j", j=64)
            for j in range(64):
                pb = ps.tile([64, 8], f32, tag="ps")
                nc.tensor.matmul(pb[:], h3[:, :, j], cw2m[:], start=True, stop=True)
                nc.vector.tensor_copy(b3[:, :, j], pb[:])

            # ---- phase 2: QKV mixing -> psum_q/k/v [s, (jm, h, d)] ----
            wq3 = wqrm.rearrange("c (o m) -> c o m", m=4)
            wi3 = wqim.rearrange("c (o m) -> c o m", m=4)
            win3 = wqin.rearrange("c (o m) -> c o m", m=4)
            pq = psl.tile([64, 512], f32, tag="psq")
            pk = psl.tile([64, 512], f32, tag="psk")
            pv = psl.tile([64, 512], f32, tag="psv")
            for m in range(4):
                lR = XR[:, 64 * m:64 * (m + 1)]
                lI = XI[:, 64 * m:64 * (m + 1)]
                for dst, o0 in ((pq, 0), (pk, 64), (pv, 128)):
                    wR = wq3[:, o0:o0 + 64, m]
                    wI = wi3[:, o0:o0 + 64, m]
                    wIn = win3[:, o0:o0 + 64, m]
                    blk = dst[:, 64 * (2 * m):64 * (2 * m) + 64]
                    nc.tensor.matmul(blk, lR, wR, start=True, stop=False)
                    nc.tensor.matmul(blk, lI, wIn, start=False, stop=True)
                    blk = dst[:, 64 * (2 * m + 1):64 * (2 * m + 1) + 64]
                    nc.tensor.matmul(blk, lR, wI, start=True, stop=False)
                    nc.tensor.matmul(blk, lI, wR, start=False, stop=True)
            nc.vector.tensor_copy(Q_sb[:], pq[:])
            nc.scalar.copy(K_sb[:], pk[:])
            nc.vector.tensor_copy(V_sb[:], pv[:])

            # ---- attention per head ----
            q4 = Q_sb.rearrange("s (j h d) -> s j h d", h=8, d=8)
            k4 = K_sb.rearrange("s (j h d) -> s j h d", h=8, d=8)
            v4 = V_sb.rearrange("s (j h d) -> s j h d", h=8, d=8)
            o4 = O_all.rearrange("s (j h d) -> s j h d", h=8, d=8)
            for h in range(8):
                qhs = sm_pool.tile([64, 64], dt_mid, tag="qhs")
                nc.vector.tensor_copy(qhs[:], q4[:, :, h, :])
                tid2 = identb if dt_mid != f32 else ident
                ptq = ps.tile([64, 64], dt_mid, tag="ps")
                nc.tensor.transpose(ptq[:], qhs[:], tid2[:64, :64])
                qh = sm_pool.tile([64, 64], dt_mid, tag="qh")
                nc.vector.tensor_scalar_mul(qh[:], ptq[:], gcol[:])
                khs = sm_pool.tile([64, 64], dt_mid, tag="khs")
                nc.scalar.copy(khs[:], k4[:, :, h, :])
                ptk = ps.tile([64, 64], dt_mid, tag="ps")
                nc.tensor.transpose(ptk[:], khs[:], tid2[:64, :64])
                kh = sm_pool.tile([64, 64], dt_mid, tag="kh")
                nc.scalar.copy(kh[:], ptk[:])
                pss = ps.tile([64, 64], f32, tag="ps")
                nc.tensor.matmul(pss[:], qh[:], kh[:], start=True, stop=True)
                ex = sm_pool.tile([64, 64], f32, tag="ex")
                sc = sm_pool.tile([64, 64], f32, tag="sc")
                nc.vector.tensor_add(sc[:], pss[:], bias_sb[:, 64 * h:64 * h + 64])
                nc.scalar.activation(ex[:], sc[:], Exp)
                se = sm_pool.tile([64, 1], f32, tag="se")
                nc.vector.reduce_sum(se[:], ex[:], axis=mybir.AxisListType.X)
                ri = sm_pool.tile([64, 1], f32, tag="ri")
                nc.vector.reciprocal(ri[:], se[:])
                an = sm_pool.tile([64, 64], dt_mid, tag="an")
                nc.vector.tensor_scalar_mul(an[:], ex[:], ri[:])
                pat = ps.tile([64, 64], dt_mid, tag="ps")
                nc.tensor.transpose(pat[:], an[:], tid2[:64, :64])
                at = sm_pool.tile([64, 64], dt_mid, tag="at")
                nc.scalar.copy(at[:], pat[:])
                vh = sm_pool.tile([64, 64], dt_mid, tag="vh")
                nc.vector.tensor_copy(vh[:], v4[:, :, h, :])
                po = ps.tile([64, 64], f32, tag="ps")
                nc.tensor.matmul(po[:], at[:], vh[:], start=True, stop=True)
                nc.vector.tensor_copy(o4[:, :, h, :], po[:])

            # ---- phase 6: T8 scale, transpose, w_out mixing ----
            O_scm = O_sc if dt_mid == f32 else singles.tile([64, 512], dt_mid, tag="Oscm")
            nc.vector.tensor_mul(O_scm[:], O_all[:], t8rep[:])
            xor3 = XOR.rearrange("c (m s) -> c m s", s=64)
            xoi3 = XOI.rearrange("c (m s) -> c m s", s=64)
            for jm in range(8):
                pt = ps.tile([64, 64], dt_mid, tag="ps")
                tid3 = identb if dt_mid != f32 else ident
                nc.tensor.transpose(pt[:], O_scm[:, 64 * jm:64 * (jm + 1)],
                                    tid3[:64, :64])
                dst3 = xor3 if jm % 2 == 0 else xoi3
                if jm % 2 == 0:
                    nc.vector.tensor_copy(dst3[:, jm // 2, :], pt[:])
                else:
                    nc.scalar.copy(dst3[:, jm // 2, :], pt[:])
            wo3 = worm.rearrange("c (o m) -> c o m", m=4)
            woi3_ = woim.rearrange("c (o m) -> c o m", m=4)
            woin3 = woin.rearrange("c (o m) -> c o m", m=4)
            pf = psl.tile([64, 512], f32, tag="psq")
            for m in range(4):
                rR = XOR[:, 64 * m:64 * (m + 1)]
                rI = XOI[:, 64 * m:64 * (m + 1)]
                wR = wo3[:, :, m]
                wI = woi3_[:, :, m]
                wIn = woin3[:, :, m]
                blk = pf[:, 64 * (2 * m):64 * (2 * m) + 64]
                nc.tensor.matmul(blk, wR, rR, start=True, stop=False)
                nc.tensor.matmul(blk, wIn, rI, start=False, stop=True)
                blk = pf[:, 64 * (2 * m + 1):64 * (2 * m + 1) + 64]
                nc.tensor.matmul(blk, wI, rR, start=True, stop=False)
                nc.tensor.matmul(blk, wR, rI, start=False, stop=True)
            nc.vector.tensor_copy(F_sb[:], pf[:])

            # ---- build U_fT [8, rows] ----
            f3 = F_sb.rearrange("c (j s) -> c j s", s=64)
            for s in range(64):
                pu = ps.tile([8, 64], f32, tag="ps")
                nc.tensor.transpose(pu[:], f3[:, :, s], ident[:64, :64])
                if s % 2 == 0:
                    nc.vector.tensor_copy(U_fT[:, 64 * s:64 * (s + 1)], pu[:])
                else:
                    nc.scalar.copy(U_fT[:, 64 * s:64 * (s + 1)], pu[:])

            # ---- stage 7: y = U_fT^T @ Bas, stream out ----
            for t in range(32):
                lh = U_fT[:, 128 * t:128 * (t + 1)]
                py1 = ps.tile([128, 512], f32, tag="ps")
                nc.tensor.matmul(py1[:], lh, basb[:, :512], start=True, stop=True)
                py2 = ps.tile([128, 512], f32, tag="ps")
                nc.tensor.matmul(py2[:], lh, basb[:, 512:], start=True, stop=True)
                y_sb = y_pool.tile([128, 1024], f32, tag="y_sb")
                nc.vector.tensor_copy(y_sb[:, :512], py1[:])
                nc.scalar.copy(y_sb[:, 512:], py2[:])
                nc.sync.dma_start(out=y_out[128 * t:128 * (t + 1), :], in_=y_sb[:])
    nc.finalize()
    return nc


_NC_CACHE = {}


def kernel(**inputs) -> np.ndarray:
    from concourse.bass_utils import run_bass_kernel_spmd

    seq = np.asarray(inputs["seq"], dtype=np.float32)
    assert seq.shape == (B, S, C, H, W)
    e8c, bas, relT, gcol, t8pat = _constants()

    if DT_BIG not in _NC_CACHE:
        _NC_CACHE[DT_BIG] = _build(DT_BIG)
    nc = _NC_CACHE[DT_BIG]

    common = {
        "wqr": np.ascontiguousarray(np.asarray(inputs["w_qkv_r"], np.float32).reshape(64, 768)),
        "wqi": np.ascontiguousarray(np.asarray(inputs["w_qkv_i"], np.float32).reshape(64, 768)),
        "wor": np.ascontiguousarray(np.asarray(inputs["w_out_r"], np.float32).reshape(64, 256)),
        "woi": np.ascontiguousarray(np.asarray(inputs["w_out_i"], np.float32).reshape(64, 256)),
        "cw1": np.asarray(inputs["cpb_w1"], np.float32),
        "cb1": np.asarray(inputs["cpb_b1"], np.float32).reshape(64, 1),
        "cw2": np.asarray(inputs["cpb_w2"], np.float32),
        "e8c": e8c, "bas": bas, "relT": relT, "gcol": gcol, "t8pat": t8pat,
    }
    in_maps = []
    for b in range(NCORES):
        m = dict(common)
        m["x"] = np.ascontiguousarray(seq[b].reshape(4096, 1024))
        in_maps.append(m)

    res = run_bass_kernel_spmd(nc, in_maps, list(range(NCORES)))
    out = np.stack([res.results[b]["y"].reshape(S, C, H, W) for b in range(NCORES)])
    return out.astype(np.float32)

